# revision 9
# baseline (speedup 1.0000x reference)
"""BitNet attention TRN2 kernel: 8-way SPMD (2 heads/core, tokens sharded 8-way).

v3: batch-pipelined attention with per-batch-half AllToAll.

Per core c (tokens [c*L,(c+1)*L) of the flattened [B*T, D] activations,
heads {2c, 2c+1}):
  A) x_had = x @ H in fp16 (fp32 accumulate); per-token int8 absmax quant;
     PE-transpose; y.T AllGather split in two token-halves, the first fired
     mid-phase so it overlaps the remaining tiles; absmax AllGather.
  B) int8 -> f16 convert of gathered y.T (DVE/ACT split); Q/K/V projections
     per batch: batch-0 QKV -> attention(batch 0) -> AllToAll(0) ->
     batch-1 QKV (fills PE slack under batch-0's exp-bound attention) ->
     attention(batch 1) with the final-quant work for half 0 scheduled into
     its PE slack -> AllToAll(1) -> final-quant half 1.
  C) Attention: S.T = Ks.T^T @ Qs.T (row-packed head pairs), exp on ACT,
     out.T = [V | 1]^T @ expS.T (ones column = softmax denominator) into one
     packed PSUM accumulator, drain to SBUF, normalize, PE-transpose to
     token-major f16, deposit striped by destination core.
  D) Per batch-half: gather own 256 tokens x 1024 dims (token-major, no
     transposes needed), absmax quant, y2.T via PE-transpose,
     z = y2 @ Wo_u.T, per-token dequant. Core c owns tokens
     [c*L/2,(c+1)*L/2) of batch 0 and [BT/2 + c*L/2, ...) of batch 1.

Host side quantizes weights to ternary (fp16-exact), transposes x, and
reassembles z from the two per-core halves.
"""
import sys

if "/opt/trn_rl_repo" not in sys.path:
    sys.path.insert(0, "/opt/trn_rl_repo")

import numpy as np

P = 128
D = 1024
NH = 16
DH = 64
B = 2
N_CORES = 8
MAGIC = 12582912.0  # 1.5 * 2**23: fp32 round-to-nearest-int via add/sub

_BUILD_CACHE = {}


def _build(T):
    import concourse.bass as bass  # noqa: F401
    import concourse.mybir as mybir
    import concourse.tile as tile
    from concourse import bacc
    from concourse.masks import make_identity

    f16 = mybir.dt.float16
    f32 = mybir.dt.float32
    i8 = mybir.dt.int8
    Exp = mybir.ActivationFunctionType.Exp
    mult = mybir.AluOpType.mult
    add = mybir.AluOpType.add
    subtract = mybir.AluOpType.subtract
    X = mybir.AxisListType.X

    BT = B * T
    L = BT // N_CORES          # tokens per core (phase A sharding)
    L2 = L // 2                # tokens per core per batch (phase D sharding)
    NT = L // P                # local token tiles
    NTH = L2 // P              # token tiles per D-half
    DK = D // P                # contraction chunks
    QB = 512                   # query block
    NQB = T // QB              # query blocks per batch
    NKT = T // P               # key tiles per batch
    VT = BT // P               # global token tiles (for V)
    TBW = 256                  # QK projection block width
    GROUPS = [list(range(N_CORES))]
    assert NTH >= 1, "v3 needs T >= 1024"

    nc = bacc.Bacc("TRN2", target_bir_lowering=False, debug=False,
                   num_devices=N_CORES)

    # I/O
    xT = nc.dram_tensor("xT", [D, L], f16, kind="ExternalInput")
    Hm = nc.dram_tensor("Hm", [D, D], f16, kind="ExternalInput")
    WqT = nc.dram_tensor("WqT", [D, P], f16, kind="ExternalInput")
    WkT = nc.dram_tensor("WkT", [D, P], f16, kind="ExternalInput")
    WvT = nc.dram_tensor("WvT", [D, P], f16, kind="ExternalInput")
    WoT = nc.dram_tensor("WoT", [D, D], f16, kind="ExternalInput")
    consts = nc.dram_tensor("consts", [1, 4], f32, kind="ExternalInput")
    z = nc.dram_tensor("z", [L, D], f32, kind="ExternalOutput")

    with tile.TileContext(nc) as tc:
        cpool = tc.alloc_tile_pool(name="cpool", bufs=1)
        dram = tc.alloc_tile_pool(name="dram", bufs=1, space="DRAM")

        ident16 = cpool.tile([P, P], f16)
        make_identity(nc, ident16)
        csb = cpool.tile([P, 4], f32)
        nc.sync.dma_start(csb, consts.ap().to_broadcast((P, 4)))

        # DRAM intermediates
        yT_loc0 = dram.tile([D, L // 2], i8)
        yT_loc1 = dram.tile([D, L // 2], i8)
        yT_g0 = dram.tile([N_CORES * D, L // 2], i8, addr_space="Shared")
        yT_g1 = dram.tile([N_CORES * D, L // 2], i8, addr_space="Shared")
        am_loc = dram.tile([1, L], f32)
        am_g = dram.tile([N_CORES, L], f32, addr_space="Shared")
        # token-major per-batch AllToAll: slot for peer p = [L2 tokens, 128]
        a2a_in = [dram.tile([N_CORES * L2, P], f16, name=f"a2ain{i}")
                  for i in range(B)]
        a2a_out = [dram.tile([N_CORES * L2, P], f16, name=f"a2aout{i}")
                   for i in range(B)]

        # ---------------- Phase A: x@H, quant, transpose, gather ----------
        with tc.tile_pool(name="pre", bufs=1) as pre, \
             tc.tile_pool(name="workA", bufs=3) as workA, \
             tc.tile_pool(name="psA", bufs=2, space="PSUM") as psA, \
             tc.tile_pool(name="psT", bufs=4, space="PSUM") as psT:
            sA = nc.named_scope("phaseA"); sA.__enter__()
            xsb = pre.tile([P, DK, L], f16)
            Hsb = pre.tile([P, DK, D], f16)
            x_v = xT.ap().rearrange("(o p) t -> p o t", p=P)
            H_v = Hm.ap().rearrange("(o p) d -> p o d", p=P)
            for kc in range(DK):
                nc.sync.dma_start(Hsb[:, kc], H_v[:, kc])
                nc.sync.dma_start(xsb[:, kc], x_v[:, kc])
            yT_sb = pre.tile([P, DK, L], i8)
            am_all = pre.tile([P, NT], f32)

            for tt in range(NT):
                ps = psA.tile([P, 1024], f32, tag="xh")
                for half in range(2):
                    for kc in range(DK):
                        nc.tensor.matmul(
                            ps[:, half * 512:(half + 1) * 512],
                            xsb[:, kc, tt * P:(tt + 1) * P],
                            Hsb[:, kc, half * 512:(half + 1) * 512],
                            start=(kc == 0), stop=(kc == DK - 1))
                am_t = am_all[:, tt:tt + 1]
                nc.vector.reduce_max(am_t, ps, axis=X, apply_absolute_value=True)
                nc.vector.tensor_scalar_max(am_t, am_t, 1e-5)
                rec = workA.tile([P, 1], f32, tag="rec")
                nc.vector.reciprocal(rec, am_t)
                s127 = workA.tile([P, 1], f32, tag="s127")
                nc.vector.tensor_scalar_mul(s127, rec, 127.0)
                tmp = workA.tile([P, 1024], f32, tag="tmpA")
                nc.vector.tensor_scalar(tmp, ps, s127, MAGIC, mult, add)
                y_t = workA.tile([P, 1024], f16, tag="yA")
                nc.vector.tensor_scalar(y_t, tmp, MAGIC, None, subtract)
                for kc in range(DK):
                    pst = psT.tile([P, P], f16, tag="trA")
                    nc.tensor.transpose(pst, y_t[:, kc * P:(kc + 1) * P], ident16)
                    nc.vector.tensor_copy(yT_sb[:, kc, tt * P:(tt + 1) * P], pst)
                if tt == max(NT // 2, 1) - 1:
                    # first token-half gathered while later tiles compute
                    nc.sync.dma_start(
                        yT_loc0.rearrange("(o p) t -> p o t", p=P),
                        yT_sb[:, :, 0:L // 2])
                    nc.gpsimd.collective_compute(
                        "AllGather", mybir.AluOpType.bypass,
                        replica_groups=GROUPS,
                        ins=[yT_loc0.opt()], outs=[yT_g0.opt()])
                if tt == NT - 1:
                    nc.sync.dma_start(am_loc.rearrange("1 (t p) -> p t", p=P),
                                      am_all)
                    nc.gpsimd.collective_compute(
                        "AllGather", mybir.AluOpType.bypass,
                        replica_groups=GROUPS,
                        ins=[am_loc.opt()], outs=[am_g.opt()])
                    nc.sync.dma_start(
                        yT_loc1.rearrange("(o p) t -> p o t", p=P),
                        yT_sb[:, :, L // 2:L])
                    nc.gpsimd.collective_compute(
                        "AllGather", mybir.AluOpType.bypass,
                        replica_groups=GROUPS,
                        ins=[yT_loc1.opt()], outs=[yT_g1.opt()])
            sA.__exit__(None, None, None)

        # ---------------- Phases B/C/D interleaved ------------------------
        fin = tc.alloc_tile_pool(name="fin", bufs=1)
        attn = tc.alloc_tile_pool(name="attn", bufs=1)
        QsT = attn.tile([P, BT], f16)
        KsT = attn.tile([P, BT], f16)
        V_A = attn.tile([P, VT, 65], f16)
        V_B = attn.tile([P, VT, 65], f16)
        wo = fin.tile([P, DK, D], f16)
        nc.sync.dma_start(wo, WoT.ap().rearrange("(o p) n -> p o n", p=P))

        with tc.tile_pool(name="gath", bufs=1) as gath, \
             tc.tile_pool(name="workB", bufs=2) as workB, \
             tc.tile_pool(name="exC", bufs=6) as exC, \
             tc.tile_pool(name="workC", bufs=2) as workC, \
             tc.tile_pool(name="workD", bufs=2) as workD, \
             tc.tile_pool(name="finD", bufs=1) as finD, \
             tc.tile_pool(name="dramC", bufs=3, space="DRAM") as dramC, \
             tc.tile_pool(name="psS", bufs=2, space="PSUM") as psS_pool, \
             tc.tile_pool(name="psO", bufs=1, space="PSUM") as psO_pool, \
             tc.tile_pool(name="psX", bufs=1, space="PSUM") as psX_pool, \
             tc.tile_pool(name="psM", bufs=1, space="PSUM") as psM_pool:
            sB = nc.named_scope("phaseB"); sB.__enter__()
            # int8 -> f16 converts, split DVE/ACT
            yTg = gath.tile([P, DK, BT], f16)
            yv0 = yT_g0.rearrange("(a o p) t -> a p o t", p=P, o=DK)
            yv1 = yT_g1.rearrange("(a o p) t -> a p o t", p=P, o=DK)
            for peer in range(N_CORES):
                for half, yv in ((0, yv0), (1, yv1)):
                    idx = peer * 2 + half
                    base = peer * L + half * (L // 2)
                    dst = yTg[:, :, base:base + L // 2]
                    if idx % 2 == 0:
                        stg = workB.tile([P, DK, L // 2], i8, tag="stgV")
                        nc.sync.dma_start(stg, yv[peer])
                        nc.vector.tensor_copy(dst, stg)
                    else:
                        stg = workB.tile([P, DK, L // 2], i8, tag="stgS")
                        nc.sync.dma_start(stg, yv[peer])
                        nc.scalar.copy(dst, stg)

            A_q = gath.tile([P, BT], f32)
            nc.sync.dma_start(
                A_q, am_g.rearrange("a l -> (a l)")[None, :].to_broadcast((P, BT)))
            nc.vector.tensor_scalar(A_q, A_q, csb[:, 0:1], None, mult)
            amt = gath.tile([P, VT], f32)
            nc.sync.dma_start(
                amt, am_g.rearrange("a (t p) -> p (a t)", p=P))
            Av = gath.tile([P, VT], f32)
            nc.vector.tensor_scalar(Av, amt, csb[:, 1:2], None, mult)
            # per-key dequant scale, folded into exp's per-partition scale
            amk = gath.tile([P, VT], f32)
            nc.vector.tensor_scalar(amk, amt, 1.0 / 127.0, None, mult)

            wq = gath.tile([P, DK, P], f16)
            nc.sync.dma_start(wq, WqT.ap().rearrange("(o p) m -> p o m", p=P))
            wk = gath.tile([P, DK, P], f16)
            nc.sync.dma_start(wk, WkT.ap().rearrange("(o p) m -> p o m", p=P))
            wv = gath.tile([P, DK, P], f16)
            nc.sync.dma_start(wv, WvT.ap().rearrange("(o p) m -> p o m", p=P))
            nc.vector.memset(V_A[:, :, 64:65], 1.0)
            nc.vector.memset(V_B[:, :, 64:65], 1.0)
            sB.__exit__(None, None, None)

            def emit_qkv(b):
                s = nc.named_scope(f"qkv{b}"); s.__enter__()
                nblk = T // TBW
                for blk in range(nblk):
                    tb = b * nblk + blk
                    sl = slice(tb * TBW, (tb + 1) * TBW)
                    psq = psM_pool.tile([P, TBW], f32, tag="misc")
                    for kc in range(DK):
                        nc.tensor.matmul(psq, wq[:, kc], yTg[:, kc, sl],
                                         start=(kc == 0), stop=(kc == DK - 1))
                    nc.vector.tensor_tensor(QsT[:, sl], psq, A_q[:, sl], mult)
                    psk = psM_pool.tile([P, TBW], f32, tag="misc")
                    for kc in range(DK):
                        nc.tensor.matmul(psk, wk[:, kc], yTg[:, kc, sl],
                                         start=(kc == 0), stop=(kc == DK - 1))
                    nc.vector.tensor_copy(KsT[:, sl], psk)
                for vt in range(b * (VT // B), (b + 1) * (VT // B)):
                    psv = psM_pool.tile([P, P], f32, tag="misc")
                    for kc in range(DK):
                        nc.tensor.matmul(psv, yTg[:, kc, vt * P:(vt + 1) * P],
                                         wv[:, kc],
                                         start=(kc == 0), stop=(kc == DK - 1))
                    nc.vector.tensor_scalar(V_A[:, vt, 0:64], psv[:, 0:64],
                                            Av[:, vt:vt + 1], None, mult)
                    nc.vector.tensor_scalar(V_B[:, vt, 0:64], psv[:, 64:128],
                                            Av[:, vt:vt + 1], None, mult)
                s.__exit__(None, None, None)

            def emit_attn(b):
                s = nc.named_scope(f"attn{b}"); s.__enter__()
                in_v = a2a_in[b].rearrange("(a t) d -> a t d", a=N_CORES)
                for qb in range(NQB):
                    q0 = b * T + qb * QB
                    po = psO_pool.tile([P, 1024], f32, tag="po")
                    for kt in range(NKT):
                        k0 = b * T + kt * P
                        ps = psS_pool.tile([P, 1024], f32, tag="S")
                        nc.tensor.matmul(ps[:, 0:512],
                                         KsT[0:64, k0:k0 + P],
                                         QsT[0:64, q0:q0 + QB],
                                         start=True, stop=True)
                        nc.tensor.matmul(ps[:, 512:1024],
                                         KsT[64:128, k0:k0 + P],
                                         QsT[64:128, q0:q0 + QB],
                                         start=True, stop=True)
                        vt = (b * T) // P + kt
                        ex = exC.tile([P, 1024], f16, tag="ex")
                        nc.scalar.activation(ex, ps, Exp,
                                             scale=amk[:, vt:vt + 1])
                        nc.tensor.matmul(po[0:65, 0:512], V_A[:, vt],
                                         ex[:, 0:512],
                                         start=(kt == 0), stop=(kt == NKT - 1))
                        nc.tensor.matmul(po[0:65, 512:1024], V_B[:, vt],
                                         ex[:, 512:1024],
                                         start=(kt == 0), stop=(kt == NKT - 1))
                    osb = workC.tile([65, 1024], f32, tag="osb")
                    nc.vector.tensor_copy(osb, po[0:65])
                    for head in range(2):
                        hs = slice(head * 512, (head + 1) * 512)
                        rrow = workC.tile([1, QB], f32, tag="rrow")
                        nc.vector.reciprocal(rrow, osb[64:65, hs])
                        rdr = dramC.tile([1, QB], f32, tag="rdr")
                        nc.sync.dma_start(rdr, rrow)
                        rbc = workC.tile([64, QB], f32, tag="rbc")
                        nc.sync.dma_start(rbc, rdr.to_broadcast((64, QB)))
                        onrm = workC.tile([64, QB], f16, tag="onrm")
                        nc.vector.tensor_tensor(onrm, osb[0:64, hs], rbc, mult)
                        for j in range(QB // P):
                            pst = psX_pool.tile([P, P], f16, tag="trX")
                            nc.tensor.transpose(pst[:, 0:64],
                                                onrm[:, j * P:(j + 1) * P],
                                                ident16[0:64, 0:64])
                            tmaj = workC.tile([P, 64], f16, tag="tmaj")
                            nc.vector.tensor_copy(tmaj, pst[:, 0:64])
                            g = qb * QB + j * P
                            peer, tl = divmod(g, L2)
                            nc.sync.dma_start(
                                in_v[peer, tl:tl + P,
                                     head * 64:(head + 1) * 64],
                                tmaj)
                s.__exit__(None, None, None)

            def emit_a2a(b):
                s = nc.named_scope(f"a2a{b}"); s.__enter__()
                nc.gpsimd.collective_compute(
                    "AllToAll", mybir.AluOpType.bypass, replica_groups=GROUPS,
                    ins=[a2a_in[b].opt()], outs=[a2a_out[b].opt()])
                s.__exit__(None, None, None)

            def emit_final(h):
                s = nc.named_scope(f"fin{h}"); s.__enter__()
                out_v = a2a_out[h].rearrange("(a t) d -> a t d", a=N_CORES)
                outf = finD.tile([P, NTH, D], f16, tag=f"outf{h}")
                for a in range(N_CORES):
                    nc.sync.dma_start(
                        outf[:, :, a * P:(a + 1) * P],
                        out_v[a].rearrange("(tt p) d -> p tt d", p=P))
                y2T = finD.tile([P, DK, L2], f16, tag=f"y2T{h}")
                a2r = finD.tile([P, NTH], f32, tag=f"a2r{h}")
                for tt in range(NTH):
                    am2 = workD.tile([P, 1], f32, tag="am2")
                    nc.vector.reduce_max(am2, outf[:, tt], axis=X,
                                         apply_absolute_value=True)
                    nc.vector.tensor_scalar_max(am2, am2, 1e-5)
                    nc.vector.tensor_tensor(a2r[:, tt:tt + 1], am2,
                                            csb[:, 2:3], mult)
                    rec = workD.tile([P, 1], f32, tag="recD")
                    nc.vector.reciprocal(rec, am2)
                    s127 = workD.tile([P, 1], f32, tag="s127D")
                    nc.vector.tensor_scalar_mul(s127, rec, 127.0)
                    tmp = workD.tile([P, D], f32, tag="tmpD")
                    nc.vector.tensor_scalar(tmp, outf[:, tt], s127, MAGIC,
                                            mult, add)
                    y2 = workD.tile([P, D], f16, tag="y2")
                    nc.vector.tensor_scalar(y2, tmp, MAGIC, None, subtract)
                    for kc in range(DK):
                        pst = psX_pool.tile([P, P], f16, tag="trX")
                        nc.tensor.transpose(pst, y2[:, kc * P:(kc + 1) * P],
                                            ident16)
                        nc.vector.tensor_copy(y2T[:, kc, tt * P:(tt + 1) * P],
                                              pst)
                for tt in range(NTH):
                    for nh in range(2):
                        psz = psM_pool.tile([P, 512], f32, tag="misc")
                        for kc in range(DK):
                            nc.tensor.matmul(
                                psz, y2T[:, kc, tt * P:(tt + 1) * P],
                                wo[:, kc, nh * 512:(nh + 1) * 512],
                                start=(kc == 0), stop=(kc == DK - 1))
                        zsb = workD.tile([P, 512], f32, tag="zsb")
                        nc.vector.tensor_scalar(zsb, psz, a2r[:, tt:tt + 1],
                                                None, mult)
                        nc.sync.dma_start(
                            z.ap()[h * L2 + tt * P:h * L2 + (tt + 1) * P,
                                   nh * 512:(nh + 1) * 512],
                            zsb)
                s.__exit__(None, None, None)

            emit_qkv(0)
            emit_attn(0)
            emit_a2a(0)
            emit_qkv(1)
            emit_attn(1)
            emit_final(0)
            emit_a2a(1)
            emit_final(1)

        attn.release()
        fin.release()
        dram.release()
        cpool.release()

    nc.compile()
    return nc


def _get_nc(T):
    if T not in _BUILD_CACHE:
        _BUILD_CACHE[T] = _build(T)
    return _BUILD_CACHE[T]


def _wquant(w):
    # reference: scale = 1/clip(mean|w|,1e-5); u = clip(round(w*scale),-1,1)/scale
    scale = np.float32(1.0) / np.maximum(
        np.float32(np.mean(np.abs(w), dtype=np.float64)), np.float32(1e-5))
    u = np.clip(np.rint(w * scale), -1, 1).astype(np.float32)
    return u, np.float32(1.0) / scale  # ternary, dequant scale (= clipped mean)


def kernel(x, mask, Wq, Wk, Wv, Wo, H):
    from concourse.bass_utils import run_bass_kernel_spmd

    x = np.asarray(x, np.float32)
    Wq = np.asarray(Wq, np.float32); Wk = np.asarray(Wk, np.float32)
    Wv = np.asarray(Wv, np.float32); Wo = np.asarray(Wo, np.float32)
    H = np.asarray(H, np.float32)
    Bx, T, Dx = x.shape
    BT = Bx * T
    L = BT // N_CORES
    L2 = L // 2

    nc = _get_nc(T)

    xf = x.reshape(BT, Dx)
    x16 = xf.astype(np.float16)
    H16 = H.astype(np.float16)

    uq, cq = _wquant(Wq); uk, ck = _wquant(Wk)
    uv, cv = _wquant(Wv); uo, co = _wquant(Wo)
    uqT = np.ascontiguousarray(uq.T.astype(np.float16))
    ukT = np.ascontiguousarray(uk.T.astype(np.float16))
    uvT = np.ascontiguousarray(uv.T.astype(np.float16))
    uoT = np.ascontiguousarray(uo.T.astype(np.float16))

    c0 = np.float32(cq) * np.float32(ck) / (np.float32(np.sqrt(DH)) * np.float32(127.0))
    c1 = np.float32(cv) / np.float32(127.0)
    c2 = np.float32(co) / np.float32(127.0)
    consts = np.array([[c0, c1, c2, 0.0]], np.float32)

    in_maps = []
    for c in range(N_CORES):
        rows = slice(c * L, (c + 1) * L)
        cols = slice(c * P, (c + 1) * P)
        in_maps.append({
            "xT": np.ascontiguousarray(x16[rows].T),
            "Hm": H16,
            "WqT": np.ascontiguousarray(uqT[:, cols]),
            "WkT": np.ascontiguousarray(ukT[:, cols]),
            "WvT": np.ascontiguousarray(uvT[:, cols]),
            "WoT": uoT,
            "consts": consts,
        })

    res = run_bass_kernel_spmd(nc, in_maps, core_ids=list(range(N_CORES)))
    kernel.last_results = res
    # reassemble: core c returned rows = [batch-0 tokens c*L2..] + [batch-1 ...]
    half = BT // 2
    zf = np.empty((BT, Dx), np.float32)
    for c in range(N_CORES):
        zc = res.results[c]["z"]
        zf[c * L2:(c + 1) * L2] = zc[0:L2]
        zf[half + c * L2:half + (c + 1) * L2] = zc[L2:L]
    return zf.reshape(Bx, T, Dx).astype(np.float32)
